# revision 18
# baseline (speedup 1.0000x reference)
"""Trainium2 Bass kernel for nn_Loss_Labels_19825569038545.

Computes -mean(log_softmax(concat([syn, ant], axis=1), axis=1)) over B=2^24 rows.

Math: per row with s=syn, a=ant, d=s-a, t=|d|:
    -(log_softmax_0 + log_softmax_1) = 2*logsumexp(s,a) - s - a
                                     = t + 2*ln(1 + exp(-t))
so the loss = (1/(2B)) * [sum_i t_i + 2*sum_i ln(1 + exp(-t_i))].

Sharding: pure data parallel. The batch is split into 8 contiguous shards,
one per NeuronCore. Each core streams its 16 MiB shard (syn and ant
interleaved per tile so one DMA feeds both operands) through SBUF:
DVE computes d = s - a then t = max(-d, d) with a fused per-partition sum
(accum_out); ACT computes e = Exp(-t) then Ln(1 + e) with a fused sum.
Each core writes back a [128, 2] tile of per-partition partials
(sum_t, sum_l). The host combines 8*128 partials in float64 and divides
by 2B. S1_out only provides B and is never transferred.

Raw Bass (no TileContext): the kernel is a simple 3-engine pipeline and
explicit semaphores keep every instruction within the 1-wait ISA limit.
"""

import sys
from contextlib import ExitStack

import numpy as np

if "/opt/trn_rl_repo" not in sys.path:
    sys.path.insert(0, "/opt/trn_rl_repo")

import concourse.bass as bass
import concourse.mybir as mybir
from concourse.bass_utils import run_bass_kernel_spmd

B = 16777216
N_CORES = 8
N = B // N_CORES          # 2,097,152 elements per core
P = 128                   # SBUF partitions
F = 4096                  # free-dim tile width (per-tile DMA = 4 MiB)
NT = N // (P * F)         # 4 tiles per core

FP32 = mybir.dt.float32

_nc_cache = {}


def _build_nc(nt=NT, f=F, repeat=1):
    """Build the per-core program.

    repeat > 1 replays the whole streaming pipeline over the same input
    (benchmarking only): per-execution wall-clock deltas between two
    repeat values isolate the steady-state per-pass time from dispatch
    overhead. repeat=1 is the graded kernel.
    """
    key = (nt, f, repeat)
    if key in _nc_cache:
        return _nc_cache[key]
    nc = bass.Bass()
    # syn and ant interleaved per tile: sa[i, :, :f] = syn chunk i,
    # sa[i, :, f:] = ant chunk i.
    sa = nc.dram_tensor("sa", [nt, P, 2 * f], FP32, kind="ExternalInput")
    out = nc.dram_tensor("out", [P, 2], FP32, kind="ExternalOutput")

    with ExitStack() as ctx:
        sa_tiles = [
            ctx.enter_context(nc.sbuf_tensor(f"sa_t{i}", [P, 2 * f], FP32))
            for i in range(nt)
        ]
        w_tiles = [
            ctx.enter_context(nc.sbuf_tensor(f"w_t{i}", [P, f], FP32))
            for i in range(nt)
        ]
        acc_t = ctx.enter_context(nc.sbuf_tensor("acc_t", [P, nt], FP32))
        acc_l = ctx.enter_context(nc.sbuf_tensor("acc_l", [P, nt], FP32))
        col = ctx.enter_context(nc.sbuf_tensor("col", [P, 2], FP32))
        load_sems = [
            ctx.enter_context(nc.semaphore(f"load{i}")) for i in range(nt)
        ]
        # Per-engine pipeline sems: every op incs by 1; consumers (same or
        # cross engine) wait on the producer's running count. Same-engine
        # RAW needs this too — the engine pipelines are deep.
        dve_pipe = ctx.enter_context(nc.semaphore("dve_pipe"))
        act_pipe = ctx.enter_context(nc.semaphore("act_pipe"))
        col_sem = ctx.enter_context(nc.semaphore("col_done"))
        st_sem = ctx.enter_context(nc.semaphore("store_done"))
        block = ctx.enter_context(nc.Block())

        @block.sync
        def _(sync):
            for r in range(repeat):
                for i in range(nt):
                    if r > 0:
                        # back-pressure: sub(r-1, i) must have consumed
                        # sa_tiles[i] before it is overwritten
                        sync.wait_ge(dve_pipe, 2 * ((r - 1) * nt + i) + 1)
                    sync.dma_start(out=sa_tiles[i][:], in_=sa[i]).then_inc(
                        load_sems[i], 16
                    )
            sync.wait_ge(col_sem, 2)
            sync.dma_start(out=out[:], in_=col[:]).then_inc(st_sem, 16)
            sync.wait_ge(st_sem, 16)

        @block.vector
        def _(vector):
            # DVE op k (0-based) incs dve_pipe to k+1.
            for r in range(repeat):
                for i in range(nt):
                    k = 2 * (r * nt + i)  # dve_pipe value before sub
                    vector.wait_ge(load_sems[i], 16 * (r + 1))
                    if r > 0:
                        # w_tiles[i] free once ln(r-1, i) is done
                        vector.wait_ge(act_pipe, 2 * ((r - 1) * nt + i) + 2)
                    # d = s - a
                    vector.tensor_sub(
                        out=w_tiles[i][:],
                        in0=sa_tiles[i][:, 0:f],
                        in1=sa_tiles[i][:, f : 2 * f],
                    ).then_inc(dve_pipe, 1)
                    # t = (d * -1) max d = |d|, with fused per-partition sum
                    vector.wait_ge(dve_pipe, k + 1)
                    vector.scalar_tensor_tensor(
                        out=w_tiles[i][:],
                        in0=w_tiles[i][:],
                        scalar=-1.0,
                        in1=w_tiles[i][:],
                        op0=mybir.AluOpType.mult,
                        op1=mybir.AluOpType.max,
                        accum_out=acc_t[:, i : i + 1],
                    ).then_inc(dve_pipe, 1)
            vector.wait_ge(dve_pipe, 2 * nt * repeat)
            vector.reduce_sum(
                out=col[:, 0:1], in_=acc_t[:], axis=mybir.AxisListType.X
            ).then_inc(col_sem, 1)
            vector.wait_ge(act_pipe, 2 * nt * repeat)
            vector.reduce_sum(
                out=col[:, 1:2], in_=acc_l[:], axis=mybir.AxisListType.X
            ).then_inc(col_sem, 1)

        @block.scalar
        def _(scalar):
            # ACT op k (0-based) incs act_pipe to k+1.
            for r in range(repeat):
                for i in range(nt):
                    k = 2 * (r * nt + i)  # act_pipe value before exp
                    # e = exp(-t), in place; t ready once stt(r, i) is done
                    scalar.wait_ge(dve_pipe, k + 2)
                    scalar.activation(
                        out=w_tiles[i][:],
                        in_=w_tiles[i][:],
                        func=mybir.ActivationFunctionType.Exp,
                        scale=-1.0,
                    ).then_inc(act_pipe, 1)
                    # l = ln(1 + e), in place, with fused per-partition sum
                    scalar.wait_ge(act_pipe, k + 1)
                    scalar.activation(
                        out=w_tiles[i][:],
                        in_=w_tiles[i][:],
                        func=mybir.ActivationFunctionType.Ln,
                        bias=1.0,
                        accum_out=acc_l[:, i : i + 1],
                    ).then_inc(act_pipe, 1)

    _nc_cache[key] = nc
    return nc


def _run(synonymy_score, antonymy_score, **spmd_kwargs):
    nc = _build_nc()
    syn = np.ascontiguousarray(synonymy_score).reshape(N_CORES, NT, P, F)
    ant = np.ascontiguousarray(antonymy_score).reshape(N_CORES, NT, P, F)
    sa = np.concatenate([syn, ant], axis=3)  # [N_CORES, NT, P, 2F]
    in_maps = [{"sa": sa[c]} for c in range(N_CORES)]
    r = run_bass_kernel_spmd(nc, in_maps, list(range(N_CORES)), **spmd_kwargs)
    sum_t = np.float64(0.0)
    sum_l = np.float64(0.0)
    for c in range(N_CORES):
        partials = r.results[c]["out"].astype(np.float64)
        sum_t += partials[:, 0].sum()
        sum_l += partials[:, 1].sum()
    value = np.asarray((sum_t + 2.0 * sum_l) / (2.0 * B), dtype=np.float32)
    return value, r


def kernel(S1_out, synonymy_score, antonymy_score):
    return _run(synonymy_score, antonymy_score)[0]
